# revision 47
# baseline (speedup 1.0000x reference)
"""Trainium2 Bass kernel for CustomFlashAttention (B=2, S=2048, D=2048, H=16).

Sharding over 8 NeuronCores: core c handles batch b=c//4 and head-group
hg=c%4 (4 heads of 128 dims = feature cols [hg*512,(hg+1)*512)).
Per core: QKV projections for its cols, causal flash attention for its 4
heads, partial output projection; host sums the 4 partials per batch.

All matmul operands are bf16 (measured 216.7ns per 512-col matmul vs
fp32r's 227ns at the full 2.4GHz PE clock; psum accumulation stays
fp32). bf16 halves every byte: x lives fully RESIDENT in SBUF (loaded
once, ~8MB — no second-pass reload), all weights preload up front, and
the kernel's total input DMA is ~12.5MB in one prioritized burst.
Inputs arrive host-relaid partition-major so DMA descriptors are 4KB+
contiguous runs (trigger issue time scales with descriptor count).
Operands consumed early live in small per-chunk tiles (kt ranges
[1,3,4,4,4]) because DMA completion dependencies are tile-granular.

Softmax skips the max-subtraction (scores ~N(0,1)) and defers the 1/sum
normalization to the attention output; row sums come from an all-ones
stationary matmul over the same exp(S^T) tiles used for P@V. The
attention inner loop is a flat software-pipelined (h,kt) stream: score
matmuls run L=3 ahead of the scalar-engine exp with P@V + rowsum
trailing, so the PE never waits on exp. Causal diagonal tiles compute
only columns >= delta (bf16 has no narrow-matmul penalty) and mask
in-place in PSUM, only the 128 partially-masked columns.
PSUM: passes use 8 banks (pass B alternates pp0-3/pp4-7 per tb);
attention uses score=4 + o=2 + shared(rowsum+outproj)=2.
"""

import os
import numpy as np
import ml_dtypes

import concourse.bacc as bacc
import concourse.mybir as mybir
import concourse.tile as tile
from concourse.bass_utils import run_bass_kernel_spmd

B = 2
S = 2048
D = 2048
H_PER_CORE = 4
DC = 512          # feature cols per core (4 heads * 128)
HD = 128          # head dim
P = 128
TB = 512          # token block
N_TB = S // TB    # 4
N_KT = S // P     # 16 (128-wide k/token tiles)
FP32 = mybir.dt.float32
BF16 = mybir.dt.bfloat16
NEG = -30000.0
L = 3             # attention pipeline lookahead

W_CHUNKS = [(0, 1), (1, 4), (4, 8), (8, 12), (12, 16)]

LAST_RESULTS = None  # BassKernelResults from the most recent run (for test.py)


def build_bass(causal: bool):
    nc = bacc.Bacc(None, target_bir_lowering=False, debug=False)

    # host-relaid inputs: partition-major, contiguous multi-kt runs
    x_d = nc.dram_tensor("xL", [P, N_TB, N_KT, TB], BF16,
                         kind="ExternalInput")          # [p, tb, ko, t]
    wq_d = nc.dram_tensor("wqL", [P, N_KT, DC], BF16, kind="ExternalInput")
    wk_d = nc.dram_tensor("wkL", [P, N_KT, DC], BF16, kind="ExternalInput")
    wv_d = nc.dram_tensor("wvL", [P, N_KT, DC], BF16, kind="ExternalInput")
    wo_d = nc.dram_tensor("woL", [P, H_PER_CORE, 4, DC], BF16,
                          kind="ExternalInput")         # [p, h, nb, n]
    g_d = nc.dram_tensor("gmask", [P, 896], FP32, kind="ExternalInput")
    out_d = nc.dram_tensor("out", [S, D], FP32, kind="ExternalOutput")

    with tile.TileContext(nc) as tc:
        with tc.tile_pool(name="persist", bufs=1) as persist:
            qt_s = persist.tile([P, H_PER_CORE, S], BF16, tag="qt")
            kt_s = persist.tile([P, H_PER_CORE, S], BF16, tag="kt")
            v_s = persist.tile([P, N_KT, DC], BF16, tag="v")
            wk_s = persist.tile([P, N_KT, DC], BF16, tag="wk")
            wo_s = persist.tile([P, H_PER_CORE, 4, DC], BF16, tag="wo")

            ones_f = persist.tile([P, P], FP32, tag="ones_f")
            nc.any.memset(ones_f[:], 1.0)
            ones_s = persist.tile([P, P], BF16, tag="ones")
            nc.vector.tensor_copy(out=ones_s[:], in_=ones_f[:])

            # chunked tiles for operands consumed while still loading
            def chunk_tiles(pfx, width):
                return [persist.tile([P, b - a, width], BF16,
                                     tag=f"{pfx}{ci}", name=f"{pfx}{ci}")
                        for ci, (a, b) in enumerate(W_CHUNKS)]

            def chunk_slice(tiles, kt, cols):
                for (a, b), t in zip(W_CHUNKS, tiles):
                    if a <= kt < b:
                        return t[:, kt - a, cols]
                raise AssertionError(kt)

            # x is resident but split per (tb, kt-chunk) tile: tb0's 2MB
            # arrives first so pass A starts early; later tbs stream in
            # behind tb0's ~28us of compute.
            x_t = [[persist.tile([P, b - a, TB], BF16, tag=f"x{tb}_{ci}",
                                 name=f"x{tb}_{ci}")
                    for ci, (a, b) in enumerate(W_CHUNKS)]
                   for tb in range(N_TB)]
            wq_t = chunk_tiles("wq", DC)
            wv_t = chunk_tiles("wv", DC)

            def load_w(tiles, src, ci):
                a, b = W_CHUNKS[ci]
                nc.sync.dma_start(tiles[ci][:], src[:, a:b, :])

            def load_x(tb, ci):
                a, b = W_CHUNKS[ci]
                nc.sync.dma_start(x_t[tb][ci][:], x_d[:, tb, a:b, :])

            def x_slice(tb, kt, cols):
                return chunk_slice(x_t[tb], kt, cols)

            # DMA priority order: first matmuls' operands lead; bulk after
            load_w(wq_t, wq_d, 0)
            load_x(0, 0)
            load_x(0, 1)
            load_w(wq_t, wq_d, 1)
            load_w(wv_t, wv_d, 0)
            load_w(wv_t, wv_d, 1)
            load_x(0, 2)
            load_x(0, 3)
            load_x(0, 4)
            load_w(wq_t, wq_d, 2)
            load_w(wv_t, wv_d, 2)
            for ci in range(5):
                load_x(1, ci)
            for ci in (3, 4):
                load_w(wq_t, wq_d, ci)
                load_w(wv_t, wv_d, ci)
            for ci in range(5):
                load_x(2, ci)
            nc.sync.dma_start(wk_s[:], wk_d[:])
            for ci in range(5):
                load_x(3, ci)
            if causal:
                g_s = persist.tile([P, 896], FP32, tag="g")
                nc.sync.dma_start(g_s[:], g_d[:])
            nc.sync.dma_start(wo_s[:], wo_d[:])

            with tc.tile_pool(name="pspr", bufs=1, space="PSUM") as pspr:
                # ---- pass A: Q (transposed) + V (natural), 8 psum banks
                for tb in range(N_TB):
                    psums = [pspr.tile([P, TB], FP32, tag=f"pp{i}",
                                       name=f"pp{i}_{tb}")
                             for i in range(8)]
                    for kt in range(N_KT):
                        first, last = kt == 0, kt == N_KT - 1
                        xk = x_slice(tb, kt, slice(0, TB))
                        for h in range(H_PER_CORE):
                            nc.tensor.matmul(
                                psums[h][:],
                                chunk_slice(wq_t, kt,
                                            slice(h * HD, (h + 1) * HD)),
                                xk,
                                start=first, stop=last)
                        for tt in range(4):
                            nc.tensor.matmul(
                                psums[4 + tt][:],
                                x_slice(tb, kt, slice(tt * P, (tt + 1) * P)),
                                chunk_slice(wv_t, kt, slice(0, DC)),
                                start=first, stop=last)
                    for h in range(H_PER_CORE):
                        dst = qt_s[:, h, tb * TB:(tb + 1) * TB]
                        if h % 2 == 0:
                            nc.vector.tensor_copy(out=dst, in_=psums[h][:])
                        else:
                            nc.scalar.copy(out=dst, in_=psums[h][:])
                    for tt in range(4):
                        dst = v_s[:, tb * 4 + tt, :]
                        if tt % 2 == 0:
                            nc.vector.tensor_copy(out=dst, in_=psums[4 + tt][:])
                        else:
                            nc.scalar.copy(out=dst, in_=psums[4 + tt][:])

                # ---- pass B: K (transposed); x already resident.
                # Alternate psum bank sets pp0-3 / pp4-7 per tb so
                # evictions never stall the next tb.
                for tb in range(N_TB):
                    bs = 0 if tb % 2 == 0 else 4
                    psums = [pspr.tile([P, TB], FP32, tag=f"pp{bs + i}",
                                       name=f"pk{i}_{tb}")
                             for i in range(H_PER_CORE)]
                    for kt in range(N_KT):
                        first, last = kt == 0, kt == N_KT - 1
                        for h in range(H_PER_CORE):
                            nc.tensor.matmul(
                                psums[h][:],
                                wk_s[:, kt, h * HD:(h + 1) * HD],
                                x_slice(tb, kt, slice(0, TB)),
                                start=first, stop=last)
                    for h in range(H_PER_CORE):
                        dst = kt_s[:, h, tb * TB:(tb + 1) * TB]
                        if h % 2 == 0:
                            nc.vector.tensor_copy(out=dst, in_=psums[h][:])
                        else:
                            nc.scalar.copy(out=dst, in_=psums[h][:])

            # ---- attention + output projection ----
            # psx (bufs=2) is shared by the rowsum accumulators (n) and the
            # output-projection accumulators (po): phase-disjoint within a
            # (qb,h), so one rotation serves both. 4 + 2 + 2 = 8 banks;
            # pss=4 lets scores run L=3 ahead of exp so P@V never waits.
            with tc.tile_pool(name="pt", bufs=6) as ptpool, \
                 tc.tile_pool(name="ot", bufs=4) as otpool, \
                 tc.tile_pool(name="small", bufs=2) as smpool, \
                 tc.tile_pool(name="outsb", bufs=6) as outsb, \
                 tc.tile_pool(name="pss", bufs=4, space="PSUM") as pss, \
                 tc.tile_pool(name="pso", bufs=2, space="PSUM") as pso, \
                 tc.tile_pool(name="psx", bufs=2, space="PSUM") as psx:

                for qb in range(N_TB):
                    nkt = 4 * qb + 4 if causal else N_KT
                    stream = [(h, kt) for h in range(H_PER_CORE)
                              for kt in range(nkt)]
                    n = len(stream)
                    pt_tiles = {}
                    po_tiles = {}
                    pn_tiles = {}
                    ot_tiles = []

                    def emit_score(h, kt):
                        diag = causal and kt >= 4 * qb
                        delta = (kt - 4 * qb) * P if diag else 0
                        s0 = delta  # bf16: no narrow-matmul penalty
                        ps_s = pss.tile([P, TB], FP32, tag="s",
                                        name=f"s_{qb}_{h}_{kt}")
                        nc.tensor.matmul(
                            ps_s[:, s0:],
                            kt_s[:, h, kt * P:(kt + 1) * P],
                            qt_s[:, h, qb * TB + s0:(qb + 1) * TB],
                            start=True, stop=True)
                        if diag:
                            # mask in place in PSUM, only the 128 columns
                            # that are partially masked: queries >=
                            # delta+128 see every key of this tile.
                            nc.vector.tensor_tensor(
                                ps_s[:, delta:delta + P],
                                ps_s[:, delta:delta + P],
                                g_s[:, 384:512],
                                mybir.AluOpType.add)
                        ptile = ptpool.tile([P, TB], BF16, tag="p",
                                            name=f"p_{qb}_{h}_{kt}")
                        nc.scalar.activation(
                            ptile[:, s0:], ps_s[:, s0:],
                            mybir.ActivationFunctionType.Exp)
                        pt_tiles[(h, kt)] = (ptile, s0)

                    def emit_pv(h, kt):
                        if kt == 0:
                            po_tiles[h] = pso.tile([P, TB], FP32, tag="o",
                                                   name=f"o_{qb}_{h}")
                            pn_tiles[h] = psx.tile([P, TB], FP32, tag="po",
                                                   name=f"n_{qb}_{h}")
                        first, last = kt == 0, kt == nkt - 1
                        ptile, s0 = pt_tiles.pop((h, kt))
                        nc.tensor.matmul(
                            po_tiles[h][:, s0:],
                            v_s[:, kt, h * HD:(h + 1) * HD],
                            ptile[:, s0:],
                            start=first, stop=last)
                        nc.tensor.matmul(
                            pn_tiles[h][:, s0:],
                            ones_s[:],
                            ptile[:, s0:],
                            start=first, stop=last)
                        if last:
                            recip = smpool.tile([P, TB], FP32, tag="r",
                                                name=f"r_{qb}_{h}")
                            nc.vector.reciprocal_approx_fast(
                                out=recip[:], in_=pn_tiles[h][:])
                            ot = otpool.tile([P, TB], BF16, tag="ot",
                                             name=f"ot_{qb}_{h}")
                            nc.vector.tensor_tensor(
                                ot[:], po_tiles[h][:], recip[:],
                                mybir.AluOpType.mult)
                            ot_tiles.append(ot)

                    for i in range(n + L):
                        if i < n:
                            emit_score(*stream[i])
                        if i >= L:
                            emit_pv(*stream[i - L])

                    # output projection for this 512-token block. The
                    # accumulators rotate through the score pool (pss,
                    # 4-deep, idle during Oproj) so evictions never gate
                    # the next group.
                    for tt in range(4):
                        row0 = qb * TB + tt * P
                        for nb in range(4):
                            ps_out = pss.tile([P, TB], FP32, tag="s",
                                              name=f"po_{qb}_{tt}_{nb}")
                            for h in range(H_PER_CORE):
                                nc.tensor.matmul(
                                    ps_out[:],
                                    ot_tiles[h][:, tt * P:(tt + 1) * P],
                                    wo_s[:, h, nb, :],
                                    start=(h == 0), stop=(h == H_PER_CORE - 1))
                            ob = outsb.tile([P, TB], FP32, tag="ob",
                                            name=f"ob_{qb}_{tt}_{nb}")
                            # evict halves on DVE + scalar in parallel
                            # (both idle here) and DMA them on separate
                            # queues: halves the evict->DMA chain and the
                            # end-of-kernel tail.
                            hw_ = TB // 2
                            nc.vector.tensor_copy(out=ob[:, 0:hw_],
                                                  in_=ps_out[:, 0:hw_])
                            nc.scalar.copy(out=ob[:, hw_:],
                                           in_=ps_out[:, hw_:])
                            c0 = nb * TB
                            nc.sync.dma_start(
                                out_d[row0:row0 + P, c0:c0 + hw_],
                                ob[:, 0:hw_])
                            nc.sync.dma_start(
                                out_d[row0:row0 + P, c0 + hw_:c0 + TB],
                                ob[:, hw_:])

    nc.compile()
    return nc


_BASS_CACHE = {}


def kernel(x, w_q, w_k, w_v, w_o, causal):
    global LAST_RESULTS
    x = np.asarray(x, dtype=np.float32)
    w_q = np.asarray(w_q, dtype=np.float32)
    w_k = np.asarray(w_k, dtype=np.float32)
    w_v = np.asarray(w_v, dtype=np.float32)
    w_o = np.asarray(w_o, dtype=np.float32)
    is_causal = bool(int(causal))

    if is_causal not in _BASS_CACHE:
        _BASS_CACHE[is_causal] = build_bass(is_causal)
    nc = _BASS_CACHE[is_causal]

    scale = np.float32(1.0 / np.sqrt(HD))
    g = np.zeros((P, 896), dtype=np.float32)
    ii = np.arange(P)[:, None]
    uu = np.arange(896)[None, :]
    g[uu < ii + 384] = NEG

    bf16 = ml_dtypes.bfloat16

    # host relayouts: partition-major with contiguous multi-kt runs so
    # device DMA descriptors are 4KB+ (see build_bass)
    def x_layout(xb):  # [S, D] -> [p, tb, ko, t]
        return np.ascontiguousarray(
            xb.reshape(N_TB, TB, N_KT, P).transpose(3, 0, 2, 1).astype(bf16))

    def w_layout(wT):  # [D, DC] -> [p, ko, m]
        return np.ascontiguousarray(
            wT.reshape(N_KT, P, DC).transpose(1, 0, 2).astype(bf16))

    def wo_layout(woT):  # [DC, D] -> [p, h, nb, n]
        return np.ascontiguousarray(
            woT.reshape(H_PER_CORE, P, 4, TB).transpose(1, 0, 2, 3)
            .astype(bf16))

    xL = [x_layout(x[b]) for b in range(B)]
    in_maps = []
    for c in range(8):
        b, hg = divmod(c, 4)
        cols = slice(hg * DC, (hg + 1) * DC)
        in_maps.append({
            "xL": xL[b],
            "wqL": w_layout(w_q[cols, :].T * scale),
            "wkL": w_layout(w_k[cols, :].T),
            "wvL": w_layout(w_v[cols, :].T),
            "woL": wo_layout(w_o[:, cols].T),
            "gmask": g,
        })

    trace = bool(os.environ.get("KERNEL_TRACE"))
    try:
        res = run_bass_kernel_spmd(nc, in_maps, list(range(8)), trace=trace)
    except Exception:
        if not trace:
            raise
        res = run_bass_kernel_spmd(nc, in_maps, list(range(8)), trace=False)
    LAST_RESULTS = res

    out = np.zeros((B, S, D), dtype=np.float32)
    for c in range(8):
        b = c // 4
        out[b] += res.results[c]["out"]
    return out


# revision 48
# speedup vs baseline: 1.0636x; 1.0636x over previous
"""Trainium2 Bass kernel for CustomFlashAttention (B=2, S=2048, D=2048, H=16).

Sharding over 8 NeuronCores: core c handles batch b=c//4 and head-group
hg=c%4 (4 heads of 128 dims = feature cols [hg*512,(hg+1)*512)).
Per core: QKV projections for its cols, causal flash attention for its 4
heads, partial output projection; host sums the 4 partials per batch.

All matmul operands are bf16 (measured 216.7ns per 512-col matmul vs
fp32r's 227ns at the full 2.4GHz PE clock; psum accumulation stays
fp32). bf16 halves every byte: x lives fully RESIDENT in SBUF (loaded
once, ~8MB — no second-pass reload), all weights preload up front, and
the kernel's total input DMA is ~12.5MB in one prioritized burst.
Inputs arrive host-relaid partition-major so DMA descriptors are 4KB+
contiguous runs (trigger issue time scales with descriptor count).
Operands consumed early live in small per-chunk tiles (kt ranges
[1,3,4,4,4]) because DMA completion dependencies are tile-granular.

Softmax skips the max-subtraction (scores ~N(0,1)) and defers the 1/sum
normalization to the attention output; row sums come from an all-ones
stationary matmul over the same exp(S^T) tiles used for P@V. The
attention inner loop is a flat software-pipelined (h,kt) stream: score
matmuls run L=3 ahead of the scalar-engine exp with P@V + rowsum
trailing, so the PE never waits on exp. Causal diagonal tiles compute
only columns >= delta (bf16 has no narrow-matmul penalty) and mask
in-place in PSUM, only the 128 partially-masked columns.
PSUM: passes use 8 banks (pass B alternates pp0-3/pp4-7 per tb);
attention uses score=4 + o=2 + shared(rowsum+outproj)=2.
"""

import os
import numpy as np
import ml_dtypes

import concourse.bacc as bacc
import concourse.mybir as mybir
import concourse.tile as tile
from concourse.bass_utils import run_bass_kernel_spmd

B = 2
S = 2048
D = 2048
H_PER_CORE = 4
DC = 512          # feature cols per core (4 heads * 128)
HD = 128          # head dim
P = 128
TB = 512          # token block
N_TB = S // TB    # 4
N_KT = S // P     # 16 (128-wide k/token tiles)
FP32 = mybir.dt.float32
BF16 = mybir.dt.bfloat16
NEG = -30000.0
L = 3             # attention pipeline lookahead

W_CHUNKS = [(0, 1), (1, 4), (4, 8), (8, 12), (12, 16)]

LAST_RESULTS = None  # BassKernelResults from the most recent run (for test.py)


def build_bass(causal: bool):
    nc = bacc.Bacc(None, target_bir_lowering=False, debug=False)

    # host-relaid inputs: partition-major, contiguous multi-kt runs
    x_d = nc.dram_tensor("xL", [P, N_TB, N_KT, TB], BF16,
                         kind="ExternalInput")          # [p, tb, ko, t]
    wq_d = nc.dram_tensor("wqL", [P, N_KT, DC], BF16, kind="ExternalInput")
    wk_d = nc.dram_tensor("wkL", [P, N_KT, DC], BF16, kind="ExternalInput")
    wv_d = nc.dram_tensor("wvL", [P, N_KT, DC], BF16, kind="ExternalInput")
    wo_d = nc.dram_tensor("woL", [P, H_PER_CORE, 4, DC], BF16,
                          kind="ExternalInput")         # [p, h, nb, n]
    g_d = nc.dram_tensor("gmask", [P, 896], FP32, kind="ExternalInput")
    out_d = nc.dram_tensor("out", [S, D], FP32, kind="ExternalOutput")

    with tile.TileContext(nc) as tc:
        with tc.tile_pool(name="persist", bufs=1) as persist:
            qt_s = persist.tile([P, H_PER_CORE, S], BF16, tag="qt")
            kt_s = persist.tile([P, H_PER_CORE, S], BF16, tag="kt")
            v_s = persist.tile([P, N_KT, DC], BF16, tag="v")
            wk_s = persist.tile([P, N_KT, DC], BF16, tag="wk")
            wo_s = persist.tile([P, H_PER_CORE, 4, DC], BF16, tag="wo")

            ones_f = persist.tile([P, P], FP32, tag="ones_f")
            nc.any.memset(ones_f[:], 1.0)
            ones_s = persist.tile([P, P], BF16, tag="ones")
            nc.vector.tensor_copy(out=ones_s[:], in_=ones_f[:])

            # chunked tiles for operands consumed while still loading
            def chunk_tiles(pfx, width):
                return [persist.tile([P, b - a, width], BF16,
                                     tag=f"{pfx}{ci}", name=f"{pfx}{ci}")
                        for ci, (a, b) in enumerate(W_CHUNKS)]

            def chunk_slice(tiles, kt, cols):
                for (a, b), t in zip(W_CHUNKS, tiles):
                    if a <= kt < b:
                        return t[:, kt - a, cols]
                raise AssertionError(kt)

            # x is resident but split per (tb, kt-chunk) tile: tb0's 2MB
            # arrives first so pass A starts early; later tbs stream in
            # behind tb0's ~28us of compute.
            x_t = [[persist.tile([P, b - a, TB], BF16, tag=f"x{tb}_{ci}",
                                 name=f"x{tb}_{ci}")
                    for ci, (a, b) in enumerate(W_CHUNKS)]
                   for tb in range(N_TB)]
            wq_t = chunk_tiles("wq", DC)
            wv_t = chunk_tiles("wv", DC)

            def load_w(tiles, src, ci):
                a, b = W_CHUNKS[ci]
                nc.sync.dma_start(tiles[ci][:], src[:, a:b, :])

            def load_x(tb, ci):
                a, b = W_CHUNKS[ci]
                nc.sync.dma_start(x_t[tb][ci][:], x_d[:, tb, a:b, :])

            def x_slice(tb, kt, cols):
                return chunk_slice(x_t[tb], kt, cols)

            # DMA priority order: first matmuls' operands lead; bulk after
            load_w(wq_t, wq_d, 0)
            load_x(0, 0)
            load_x(0, 1)
            load_w(wq_t, wq_d, 1)
            load_w(wv_t, wv_d, 0)
            load_w(wv_t, wv_d, 1)
            load_x(0, 2)
            load_x(0, 3)
            load_x(0, 4)
            load_w(wq_t, wq_d, 2)
            load_w(wv_t, wv_d, 2)
            for ci in range(5):
                load_x(1, ci)
            for ci in (3, 4):
                load_w(wq_t, wq_d, ci)
                load_w(wv_t, wv_d, ci)
            for ci in range(5):
                load_x(2, ci)
            nc.sync.dma_start(wk_s[:], wk_d[:])
            for ci in range(5):
                load_x(3, ci)
            if causal:
                g_s = persist.tile([P, 896], FP32, tag="g")
                nc.sync.dma_start(g_s[:], g_d[:])
            nc.sync.dma_start(wo_s[:], wo_d[:])

            with tc.tile_pool(name="pspr", bufs=1, space="PSUM") as pspr:
                # ---- pass A: Q (transposed) + V (natural), 8 psum banks
                for tb in range(N_TB):
                    psums = [pspr.tile([P, TB], FP32, tag=f"pp{i}",
                                       name=f"pp{i}_{tb}")
                             for i in range(8)]
                    for kt in range(N_KT):
                        first, last = kt == 0, kt == N_KT - 1
                        xk = x_slice(tb, kt, slice(0, TB))
                        for h in range(H_PER_CORE):
                            nc.tensor.matmul(
                                psums[h][:],
                                chunk_slice(wq_t, kt,
                                            slice(h * HD, (h + 1) * HD)),
                                xk,
                                start=first, stop=last)
                        for tt in range(4):
                            nc.tensor.matmul(
                                psums[4 + tt][:],
                                x_slice(tb, kt, slice(tt * P, (tt + 1) * P)),
                                chunk_slice(wv_t, kt, slice(0, DC)),
                                start=first, stop=last)
                    for h in range(H_PER_CORE):
                        dst = qt_s[:, h, tb * TB:(tb + 1) * TB]
                        if h % 2 == 0:
                            nc.vector.tensor_copy(out=dst, in_=psums[h][:])
                        else:
                            nc.scalar.copy(out=dst, in_=psums[h][:])
                    for tt in range(4):
                        dst = v_s[:, tb * 4 + tt, :]
                        if tt % 2 == 0:
                            nc.vector.tensor_copy(out=dst, in_=psums[4 + tt][:])
                        else:
                            nc.scalar.copy(out=dst, in_=psums[4 + tt][:])

                # ---- pass B: K (transposed); x already resident.
                # Alternate psum bank sets pp0-3 / pp4-7 per tb so
                # evictions never stall the next tb.
                for tb in range(N_TB):
                    bs = 0 if tb % 2 == 0 else 4
                    psums = [pspr.tile([P, TB], FP32, tag=f"pp{bs + i}",
                                       name=f"pk{i}_{tb}")
                             for i in range(H_PER_CORE)]
                    for kt in range(N_KT):
                        first, last = kt == 0, kt == N_KT - 1
                        for h in range(H_PER_CORE):
                            nc.tensor.matmul(
                                psums[h][:],
                                wk_s[:, kt, h * HD:(h + 1) * HD],
                                x_slice(tb, kt, slice(0, TB)),
                                start=first, stop=last)
                    for h in range(H_PER_CORE):
                        dst = kt_s[:, h, tb * TB:(tb + 1) * TB]
                        if h % 2 == 0:
                            nc.vector.tensor_copy(out=dst, in_=psums[h][:])
                        else:
                            nc.scalar.copy(out=dst, in_=psums[h][:])

            # ---- attention + output projection ----
            # psx (bufs=2) is shared by the rowsum accumulators (n) and the
            # output-projection accumulators (po): phase-disjoint within a
            # (qb,h), so one rotation serves both. 4 + 2 + 2 = 8 banks;
            # pss=4 lets scores run L=3 ahead of exp so P@V never waits.
            with tc.tile_pool(name="pt", bufs=6) as ptpool, \
                 tc.tile_pool(name="ot", bufs=4) as otpool, \
                 tc.tile_pool(name="small", bufs=2) as smpool, \
                 tc.tile_pool(name="outsb", bufs=6) as outsb, \
                 tc.tile_pool(name="pss", bufs=4, space="PSUM") as pss, \
                 tc.tile_pool(name="pso", bufs=2, space="PSUM") as pso, \
                 tc.tile_pool(name="psx", bufs=2, space="PSUM") as psx:

                for qb in range(N_TB):
                    nkt = 4 * qb + 4 if causal else N_KT
                    stream = [(h, kt) for h in range(H_PER_CORE)
                              for kt in range(nkt)]
                    n = len(stream)
                    pt_tiles = {}
                    po_tiles = {}
                    pn_tiles = {}
                    ot_tiles = []

                    def emit_score(h, kt):
                        diag = causal and kt >= 4 * qb
                        delta = (kt - 4 * qb) * P if diag else 0
                        s0 = delta  # bf16: no narrow-matmul penalty
                        ps_s = pss.tile([P, TB], FP32, tag="s",
                                        name=f"s_{qb}_{h}_{kt}")
                        nc.tensor.matmul(
                            ps_s[:, s0:],
                            kt_s[:, h, kt * P:(kt + 1) * P],
                            qt_s[:, h, qb * TB + s0:(qb + 1) * TB],
                            start=True, stop=True)
                        if diag:
                            # mask in place in PSUM, only the 128 columns
                            # that are partially masked: queries >=
                            # delta+128 see every key of this tile.
                            nc.vector.tensor_tensor(
                                ps_s[:, delta:delta + P],
                                ps_s[:, delta:delta + P],
                                g_s[:, 384:512],
                                mybir.AluOpType.add)
                        ptile = ptpool.tile([P, TB], BF16, tag="p",
                                            name=f"p_{qb}_{h}_{kt}")
                        nc.scalar.activation(
                            ptile[:, s0:], ps_s[:, s0:],
                            mybir.ActivationFunctionType.Exp)
                        pt_tiles[(h, kt)] = (ptile, s0)

                    def emit_pv(h, kt):
                        if kt == 0:
                            po_tiles[h] = pso.tile([P, TB], FP32, tag="o",
                                                   name=f"o_{qb}_{h}")
                            pn_tiles[h] = psx.tile([P, TB], FP32, tag="po",
                                                   name=f"n_{qb}_{h}")
                        first, last = kt == 0, kt == nkt - 1
                        ptile, s0 = pt_tiles.pop((h, kt))
                        nc.tensor.matmul(
                            po_tiles[h][:, s0:],
                            v_s[:, kt, h * HD:(h + 1) * HD],
                            ptile[:, s0:],
                            start=first, stop=last)
                        nc.tensor.matmul(
                            pn_tiles[h][:, s0:],
                            ones_s[:],
                            ptile[:, s0:],
                            start=first, stop=last)
                        if last:
                            recip = smpool.tile([P, TB], FP32, tag="r",
                                                name=f"r_{qb}_{h}")
                            nc.vector.reciprocal_approx_fast(
                                out=recip[:], in_=pn_tiles[h][:])
                            ot = otpool.tile([P, TB], BF16, tag="ot",
                                             name=f"ot_{qb}_{h}")
                            nc.vector.tensor_tensor(
                                ot[:], po_tiles[h][:], recip[:],
                                mybir.AluOpType.mult)
                            ot_tiles.append(ot)

                    for i in range(n + L):
                        if i < n:
                            emit_score(*stream[i])
                        if i >= L:
                            emit_pv(*stream[i - L])

                    # output projection for this 512-token block. The
                    # accumulators rotate through the score pool (pss,
                    # 4-deep, idle during Oproj) so evictions never gate
                    # the next group.
                    for tt in range(4):
                        row0 = qb * TB + tt * P
                        for nb in range(4):
                            ps_out = pss.tile([P, TB], FP32, tag="s",
                                              name=f"po_{qb}_{tt}_{nb}")
                            for h in range(H_PER_CORE):
                                nc.tensor.matmul(
                                    ps_out[:],
                                    ot_tiles[h][:, tt * P:(tt + 1) * P],
                                    wo_s[:, h, nb, :],
                                    start=(h == 0), stop=(h == H_PER_CORE - 1))
                            ob = outsb.tile([P, TB], FP32, tag="ob",
                                            name=f"ob_{qb}_{tt}_{nb}")
                            if nb % 2 == 0:
                                nc.vector.tensor_copy(out=ob[:], in_=ps_out[:])
                            else:
                                nc.scalar.copy(out=ob[:], in_=ps_out[:])
                            nc.sync.dma_start(
                                out_d[row0:row0 + P, nb * TB:(nb + 1) * TB],
                                ob[:])

    nc.compile()
    return nc


_BASS_CACHE = {}


def kernel(x, w_q, w_k, w_v, w_o, causal):
    global LAST_RESULTS
    x = np.asarray(x, dtype=np.float32)
    w_q = np.asarray(w_q, dtype=np.float32)
    w_k = np.asarray(w_k, dtype=np.float32)
    w_v = np.asarray(w_v, dtype=np.float32)
    w_o = np.asarray(w_o, dtype=np.float32)
    is_causal = bool(int(causal))

    if is_causal not in _BASS_CACHE:
        _BASS_CACHE[is_causal] = build_bass(is_causal)
    nc = _BASS_CACHE[is_causal]

    scale = np.float32(1.0 / np.sqrt(HD))
    g = np.zeros((P, 896), dtype=np.float32)
    ii = np.arange(P)[:, None]
    uu = np.arange(896)[None, :]
    g[uu < ii + 384] = NEG

    bf16 = ml_dtypes.bfloat16

    # host relayouts: partition-major with contiguous multi-kt runs so
    # device DMA descriptors are 4KB+ (see build_bass)
    def x_layout(xb):  # [S, D] -> [p, tb, ko, t]
        return np.ascontiguousarray(
            xb.reshape(N_TB, TB, N_KT, P).transpose(3, 0, 2, 1).astype(bf16))

    def w_layout(wT):  # [D, DC] -> [p, ko, m]
        return np.ascontiguousarray(
            wT.reshape(N_KT, P, DC).transpose(1, 0, 2).astype(bf16))

    def wo_layout(woT):  # [DC, D] -> [p, h, nb, n]
        return np.ascontiguousarray(
            woT.reshape(H_PER_CORE, P, 4, TB).transpose(1, 0, 2, 3)
            .astype(bf16))

    xL = [x_layout(x[b]) for b in range(B)]
    in_maps = []
    for c in range(8):
        b, hg = divmod(c, 4)
        cols = slice(hg * DC, (hg + 1) * DC)
        in_maps.append({
            "xL": xL[b],
            "wqL": w_layout(w_q[cols, :].T * scale),
            "wkL": w_layout(w_k[cols, :].T),
            "wvL": w_layout(w_v[cols, :].T),
            "woL": wo_layout(w_o[:, cols].T),
            "gmask": g,
        })

    trace = bool(os.environ.get("KERNEL_TRACE"))
    try:
        res = run_bass_kernel_spmd(nc, in_maps, list(range(8)), trace=trace)
    except Exception:
        if not trace:
            raise
        res = run_bass_kernel_spmd(nc, in_maps, list(range(8)), trace=False)
    LAST_RESULTS = res

    out = np.zeros((B, S, D), dtype=np.float32)
    for c in range(8):
        b = c // 4
        out[b] += res.results[c]["out"]
    return out
